# revision 67
# baseline (speedup 1.0000x reference)
"""Trainium2 Bass kernel for GQA attention (B=2, S=2048, D=1024, 16 q heads,
4 kv heads, head_dim 64, RoPE, causal).

Sharding: 8 cores = 2 (batch) x 4 (kv-head groups). Each core computes, for
its batch b and kv group g: the 4 query heads of group g + 1 kv head, plus the
partial output projection y_partial = attn_out_g @ wo[:, g_cols].T.  The host
unshard step sums the 4 partials per batch (the canonical all-reduce of
row-parallel TP, done on host since each core's output is already needed
host-side).

Device-side layout choices (all matmuls contract over the partition dim):
  - x is fed transposed (D on partitions) so QKV projections produce Q^T/K^T
    (head_dim on partitions, seq on free dim).  All inputs are host-packed
    into per-partition-contiguous blobs so input DMAs run at full HBM rate;
    the load order is staged so each piece lands just before first use.
  - RoPE: wq/wk rows are permuted on host so lanes 0-31 are the "real" pair
    lanes and 32-63 the "imag" lanes (the permutation cancels in Q.K^T).
    RoPE is 2 full-width DVE multiplies (m0 = ps*cos, m1 = ps*sin, both
    bf16 SBUF), then a PE matmul applies the pair-swap-with-sign
    permutation (P2 @ m1, written into the dead projection PSUM rows) and
    ONE full-width DVE add finishes: dst = m0 + P2@m1.  The matmul+add
    (rope_fin) are deferred one projection piece so the in-order PE queue
    never waits on the DVE products.
  - Scores are computed as S^T (keys on partitions, queries on free):
    lhsT = K^T block, rhs = Q^T block.  Softmax needs no max-subtraction
    (|scores/8| <~ 3), so exp runs directly on the PSUM scores; the
    denominator is produced by an extra ones-row in the V stationary
    (out row 64 of the PV matmul = sum_l P^T[l, q]).
  - S blocks are paired into 2-bank PSUM tiles so each exp ACTIVATE covers
    (128, 1024) — halves ScalarE instruction + semaphore overhead.
  - Causal mask: matmuls are only emitted for the lower-triangle blocks
    (and column-trimmed on the diagonal superblock); the 128x128 diagonal
    blocks are masked multiplicatively (tri mask, both heads per gpsimd op).
  - Normalization: ScalarE stages the denominator row to SBUF,
    reciprocal_approx_fast (base partition 0), bf16 cast, then a K=1 bf16
    broadcast matmul per head spreads 1/denom across 64 partitions; one
    DVE multiply per head normalizes.  All of it runs as deferred fillers
    one superblock later, except an eager tail on the last superblock.
  - wo runs per 128-row q chunk x 512-col half as PE fillers; output DMAs
    rotate across the sync/gpsimd/scalar queues.
  - A short run of dummy matmuls at kernel start keeps the PE busy during
    the input DMA wait so the HAM clock-gate reaches 2.4 GHz before the
    real work arrives.
"""

import sys

sys.path.insert(0, "/opt/trn_rl_repo")

from contextlib import ExitStack

import ml_dtypes
import numpy as np

import concourse.bass as bass
import concourse.mybir as mybir
import concourse.tile as tile
from concourse import bacc
from concourse.masks import make_identity

# ---------------------------------------------------------------- constants
B, S, D = 2, 2048, 1024
HD = 64
HALF = HD // 2
HKV = 4          # kv heads total
NH = 4           # q heads per core (= NREP)
KVD = HKV * HD   # 256
GO = NH * HD     # 256 output features per group
N_CORES = 8

SB = 512         # q superblock (matmul free dim)
NQS = S // SB    # 4 q superblocks
NLB = S // 128   # 16 key blocks of 128
KCH = D // 128   # 8 contraction chunks for projections
NWARM = 10       # PE warm-up matmuls during the input DMA wait

F32 = mybir.dt.float32
F32R = mybir.dt.float32r
BF16 = mybir.dt.bfloat16
SCALE = 1.0 / 8.0  # 1/sqrt(64)
EXP = mybir.ActivationFunctionType.Exp

_CACHE = {}
DEBUG_DUMPS = False  # set True (before get_nc) to add intermediate outputs


# ---------------------------------------------------------------- builder
def _enable_ldw_opt():
    """Turn on walrus's LDWEIGHTS optimization for this kernel's compile.
    The stock concourse command line pins --enable-ldw-opt=false; with ~550
    self-loading matmuls the un-hoisted weight loads cost ~60us of PE time."""
    import os
    if os.environ.get("KERNEL_LDW_OPT", "0") != "1":
        return
    import concourse.bass_utils as _bu
    if getattr(_bu, "_ldw_opt_patched", False):
        return
    _orig = _bu.run_command

    def _patched(cmd, *a, **kw):
        if isinstance(cmd, list):
            cmd = ["--enable-ldw-opt=true" if c == "--enable-ldw-opt=false"
                   else c for c in cmd]
        return _orig(cmd, *a, **kw)

    _bu.run_command = _patched
    _bu._ldw_opt_patched = True


def _build():
    _enable_ldw_opt()
    nc = bacc.Bacc("TRN2", target_bir_lowering=False, debug=False,
                   enable_asserts=False, num_devices=N_CORES)

    xt_d = nc.dram_tensor("xt", [128, NQS, KCH, SB], BF16,
                          kind="ExternalInput").ap()
    wqt_d = nc.dram_tensor("wqt", [128, KCH, GO], BF16,
                           kind="ExternalInput").ap()
    wkvt_d = nc.dram_tensor("wkvt", [128, KCH, 2 * HD], BF16,
                            kind="ExternalInput").ap()
    wot_d = nc.dram_tensor("wot", [128, 2, D], BF16, kind="ExternalInput").ap()
    cs_d = nc.dram_tensor("cs", [128, 2, S], BF16, kind="ExternalInput").ap()
    tri2_d = nc.dram_tensor("tri2", [128, 2, 128], BF16,
                            kind="ExternalInput").ap()
    p2t_d = nc.dram_tensor("p2t", [128, 128], BF16, kind="ExternalInput").ap()
    out_d = nc.dram_tensor("out", [S, D], BF16, kind="ExternalOutput").ap()
    dbg = {}
    if DEBUG_DUMPS:
        dbg["qT"] = nc.dram_tensor("dbg_qT", [128, 2, S], BF16,
                                   kind="ExternalOutput").ap()
        dbg["kT"] = nc.dram_tensor("dbg_kT", [128, S], BF16,
                                   kind="ExternalOutput").ap()
        dbg["v"] = nc.dram_tensor("dbg_v", [128, NLB, HD + 1], BF16,
                                  kind="ExternalOutput").ap()
        dbg["att"] = nc.dram_tensor("dbg_att", [128, 2, S], BF16,
                                    kind="ExternalOutput").ap()
        dbg["db"] = nc.dram_tensor("dbg_db", [32, 2, 2, SB], F32,
                                   kind="ExternalOutput").ap()

    with ExitStack() as ctx:
        tc = ctx.enter_context(tile.TileContext(nc))
        _emit(nc, tc, ctx, xt_d, wqt_d, wkvt_d, wot_d, cs_d, tri2_d, p2t_d,
              out_d, dbg)

    nc.compile()
    return nc


def _emit(nc, tc, ctx, xt_d, wqt_d, wkvt_d, wot_d, cs_d, tri2_d, p2t_d,
          out_d, dbg={}):
    perm = ctx.enter_context(tc.tile_pool(name="perm", bufs=1))
    pexp = ctx.enter_context(tc.tile_pool(name="pexp", bufs=8))
    ptmp = ctx.enter_context(tc.tile_pool(name="ptmp", bufs=5))
    pout = ctx.enter_context(tc.tile_pool(name="pout", bufs=6))
    pp_mm = ctx.enter_context(tc.tile_pool(name="ppmm", bufs=2, space="PSUM"))

    # ---------------- persistent SBUF tensors
    xt_sb = perm.tile([128, NQS, KCH, SB], BF16, tag="xt")
    wqt_sb = perm.tile([128, KCH, GO], BF16, tag="wqt")
    wkvt_sb = perm.tile([128, KCH, 2 * HD], BF16, tag="wkvt")
    wot_sb = perm.tile([128, 2, D], BF16, tag="wot")
    cs_sb = perm.tile([128, 2, S], BF16, tag="cs")      # [c|s] replicated 4x
    tri2_sb = perm.tile([128, 2, 128], BF16, tag="tri2")
    p2t_sb = perm.tile([128, 128], BF16, tag="p2t")
    ident = perm.tile([64, 64], BF16, tag="ident")
    qT_sb = perm.tile([128, 2, S], BF16, tag="qT")       # [hd|hd, mi, s]
    kT_sb = perm.tile([128, S], BF16, tag="kT")          # rows 64-127 = dup
    v_sb = perm.tile([128, NLB, HD + 1], BF16, tag="v")  # [l, lb, hd|1]
    att_sb = perm.tile([128, 2, S], BF16, tag="att")     # [o%128, o//128, s]
    ones_sb = perm.tile([32, 64], BF16, tag="ones")      # K=1 bcast stationary
    warm_sb = perm.tile([128, SB], BF16, tag="warm")

    # ---------------- PE warm-up: keep the HAM activity window busy while
    # the input DMAs land, so the first real matmuls run at 2.4 GHz.
    nc.vector.memset(warm_sb[:], 0.0)
    with tc.tile_pool(name="pwarm", bufs=2, space="PSUM") as pwarm:
        for _ in range(NWARM):
            pw = pwarm.tile([128, SB], F32, tag="warm", name="pw")
            nc.tensor.matmul(pw[:], warm_sb[:, 0:128], warm_sb[:],
                             start=True, stop=True)

    # ---------------- input DMAs (critical-path first: wkv + x chunk 0)
    # Ordered so each piece lands just before its first use: kv projection
    # (wkvt + xt0 first half), rope (cs superblock-0 columns), q projection
    # (wqt), then the rest.  wot is deferred until after the prologue so the
    # first-superblock loads get the full HBM bandwidth.
    nc.sync.dma_start(wkvt_sb[:], wkvt_d)
    nc.sync.dma_start(xt_sb[:, 0, 0:4], xt_d[:, 0, 0:4])
    nc.gpsimd.dma_start(cs_sb[:, :, 0:SB], cs_d[:, :, 0:SB])
    nc.gpsimd.dma_start(wqt_sb[:, 0:4], wqt_d[:, 0:4])
    nc.sync.dma_start(xt_sb[:, 0, 4:8], xt_d[:, 0, 4:8])
    nc.gpsimd.dma_start(wqt_sb[:, 4:8], wqt_d[:, 4:8])
    nc.sync.dma_start(xt_sb[:, 1], xt_d[:, 1])
    nc.gpsimd.dma_start(cs_sb[:, :, SB:], cs_d[:, :, SB:])
    nc.scalar.dma_start(p2t_sb[:], p2t_d)
    nc.scalar.dma_start(tri2_sb[:], tri2_d)
    make_identity(nc, ident[:])
    nc.vector.memset(ones_sb[:], 1.0)

    # ---------------- helper: RoPE on a psum projection tile
    # ps rows per 64-row head block: [real(32); imag(32)].  m0 = ps*cos,
    # m1 = ps*sin (both SBUF bf16); a PE matmul applies the pair-swap-with-
    # sign permutation (rp = P2 @ m1) and ONE full-width DVE add finishes:
    # dst = m0 + rp.  rope_mul runs with the projection piece that produced
    # ps; rope_fin is deferred one piece so the PE never waits in-queue on
    # the DVE products.
    _rpend = {}

    def rope_mul(key, ps, nrow, cols):
        m0 = ptmp.tile([128, SB], BF16, tag="ropem0", name="m0")[0:nrow]
        m1 = ptmp.tile([128, SB], BF16, tag="ropem1", name="m1")[0:nrow]
        nc.vector.tensor_mul(m0[:], ps, cs_sb[0:nrow, 0, cols])
        nc.vector.tensor_mul(m1[:], ps, cs_sb[0:nrow, 1, cols])
        _rpend[key] = (m0, m1, ps)

    def rope_fin(key, nrow, dst):
        # the permutation matmul reuses the (now dead) projection PSUM rows
        # as its output, so no extra PSUM slot is needed; the write-after-
        # read dependency on the rope multiplies orders it correctly.
        m0, m1, ps = _rpend.pop(key)
        nc.tensor.matmul(ps, p2t_sb[0:nrow, 0:nrow], m1[:],
                         start=True, stop=True)
        nc.vector.tensor_add(dst, m0[:], ps)

    # ---------------- fused pipeline: projections(s) then attention(qs=s)
    nc.vector.memset(v_sb[:, :, HD:HD + 1], 1.0)  # ones column -> denom

    _pend = {}

    def proj_q_a(si, mi, scratch=None, pspool=None):
        ps = (pspool or pp_mm).tile([128, SB], F32, tag="mm", name="psq")
        _pend[("q", si, mi)] = ps
        for kc in range(KCH // 2):
            nc.tensor.matmul(
                ps[:], wqt_sb[:, kc, mi * 128:(mi + 1) * 128],
                xt_sb[:, si, kc, :], start=(kc == 0), stop=False)

    def proj_q_b(si, mi, scratch=None, pspool=None):
        cols = slice(si * SB, (si + 1) * SB)
        ps = _pend.pop(("q", si, mi))
        for kc in range(KCH // 2, KCH):
            nc.tensor.matmul(
                ps[:], wqt_sb[:, kc, mi * 128:(mi + 1) * 128],
                xt_sb[:, si, kc, :], start=False, stop=(kc == KCH - 1))
        rope_mul(("q", si, mi), ps[:], 128, slice(si * SB, (si + 1) * SB))

    def proj_q_fin(si, mi, scratch=None, pspool=None):
        cols = slice(si * SB, (si + 1) * SB)
        rope_fin(("q", si, mi), 128, qT_sb[:, mi, cols])

    def proj_kv_a(si, scratch=None, pspool=None):
        ps = (pspool or pp_mm).tile([128, SB], F32, tag="mm", name="pskv")
        _pend[("kv", si)] = ps
        for kc in range(KCH // 2):
            nc.tensor.matmul(ps[:], wkvt_sb[:, kc, :], xt_sb[:, si, kc, :],
                             start=(kc == 0), stop=False)

    def proj_kv_b(si, scratch=None, pspool=None):
        cols = slice(si * SB, (si + 1) * SB)
        ps = _pend[("kv", si)]
        for kc in range(KCH // 2, KCH):
            nc.tensor.matmul(ps[:], wkvt_sb[:, kc, :], xt_sb[:, si, kc, :],
                             start=False, stop=(kc == KCH - 1))
        rope_mul(("kv", si), ps[0:64], 64, cols)

    def proj_kv_fin(si, scratch=None, pspool=None):
        cols = slice(si * SB, (si + 1) * SB)
        rope_fin(("kv", si), 64, kT_sb[0:64, cols])
        # duplicate K^T to partitions 64-127 for the odd-head row-tiled mms.
        # Alternate scalar/gpsimd queues: each dup gates its superblock's
        # first score matmul, and a single engine's FIFO (EXPs on scalar,
        # tri-masks on gpsimd) can delay the trigger by microseconds.
        eng = nc.scalar if si % 2 == 0 else nc.gpsimd
        eng.dma_start(kT_sb[64:128, cols], kT_sb[0:64, cols])

    def proj_v_tail(si, scratch=None, pspool=None):
        ps = _pend.pop(("kv", si))
        vt = ptmp.tile([64, SB], BF16, tag="vtstage")
        nc.scalar.copy(vt[:], ps[64:128])
        pt4 = (scratch or pp_mm).tile([128, 4, HD], BF16, tag="mm", name="pt4")
        for j in range(SB // 128):
            nc.tensor.transpose(pt4[:, j, :], vt[:, j * 128:(j + 1) * 128],
                                ident[:])
        lb0 = si * (SB // 128)
        nc.scalar.copy(v_sb[:, lb0:lb0 + 4, 0:HD], pt4[:])

    def proj_pieces(si):
        # v_tail after q0 so the DVE rope chain reaches qT(mi=0) sooner —
        # it gates the first S matmul of the superblock.  (q0-before-kv was
        # tried and regresses badly: kT_sb feeds the score-matmul stationary
        # and its rope + duplicate DMA must clear the DVE/scalar queues
        # first.)
        return [(proj_kv_a, (si,)), (proj_kv_b, (si,)),
                (proj_q_a, (si, 0)), (proj_q_b, (si, 0)),
                (proj_v_tail, (si,)), (proj_kv_fin, (si,)),
                (proj_q_fin, (si, 0)),
                (proj_q_a, (si, 1)), (proj_q_b, (si, 1)),
                (proj_q_fin, (si, 1))]

    def proj_chunk(si, scratch=None, pspool=None):
        for fn, args in proj_pieces(si):
            fn(*args, scratch, pspool)

    def wo_half(si, dh):
        """half of the output projection for one 128-row q chunk"""
        scols = slice(si * 128, (si + 1) * 128)
        if dh == 0:
            ysbs[si] = pout.tile([128, D], BF16, tag="ysb", name="ysb")
        ysb = ysbs[si]
        yp = pp_mm.tile([128, 512], F32, tag="mm", name="yp")
        for oc in range(2):
            nc.tensor.matmul(
                yp[:], att_sb[:, oc, scols],
                wot_sb[:, oc, dh * 512:(dh + 1) * 512],
                start=(oc == 0), stop=(oc == 1))
        # last superblock's copies all run on ScalarE: the tail has no EXP
        # work left while the DVE still owes the final norm chain.
        if si % 2 == 0 and si < 12:
            nc.vector.tensor_copy(ysb[:, dh * 512:(dh + 1) * 512], yp[:])
        else:
            nc.scalar.copy(ysb[:, dh * 512:(dh + 1) * 512], yp[:])
        eng = (nc.sync, nc.gpsimd, nc.scalar)[(2 * si + dh) % 3]
        eng.dma_start(out_d[scols, dh * 512:(dh + 1) * 512],
                      ysb[:, dh * 512:(dh + 1) * 512])

    ysbs = {}

    # prologue: projections for superblock 0 except the q1 pieces — those
    # run as the first attention fillers so (a) the prologue DVE rope chain
    # reaches qT(mi=0) sooner and (b) two PSUM banks stay free for the
    # first S-matmul tiles (the attention pools otherwise wait for the
    # prologue pools' last rope read before their banks recycle).
    with tc.tile_pool(name="ppro", bufs=2, space="PSUM") as ppro, \
            tc.tile_pool(name="pspro", bufs=2, space="PSUM") as pspro:
        for fn, args in proj_pieces(0)[:7]:
            fn(*args, ppro, pspro)
    # xt superblocks 2-3 ahead of wot (wot is only needed once the first
    # superblock's attention output is normalized); the wot load itself is
    # deferred into the qs=0 attention phase via a filler so its HBM traffic
    # does not compete with the xt loads.
    nc.sync.dma_start(xt_sb[:, 2], xt_d[:, 2])
    nc.sync.dma_start(xt_sb[:, 3], xt_d[:, 3])
    pp_sb = ctx.enter_context(tc.tile_pool(name="ppsb", bufs=2, space="PSUM"))
    pp_pv = ctx.enter_context(tc.tile_pool(name="pppv", bufs=1, space="PSUM"))

    def norm_mi(qs, mi, db, dbb, aus):
        # broadcast each head's 1/denom across 64 partitions with a K=1
        # bf16 matmul (fp32 moving runs as a 2-pass LOW/HIGH matmul at 4x
        # the cost), then a DVE multiply normalizes it (PSUM operands are
        # partition-unconstrained).
        qcols = slice(qs * SB, (qs + 1) * SB)
        for i in range(2):
            b0 = i * 64
            rbp = pp_mm.tile([HD, SB], F32, tag="mm", name="rbp")
            nc.tensor.matmul(rbp[:], ones_sb[0:1, 0:64],
                             dbb[0:1, mi, i, :], start=True, stop=True)
            nc.vector.tensor_mul(att_sb[b0:b0 + 64, mi, qcols],
                                 aus[mi][b0:b0 + 64, :], rbp[:])

    prev_norm = None  # (db, aus) of the previous superblock
    fillers = []       # (deadline_qs, fn, args) — popped one per group

    for qs in range(NQS):
        qcols = slice(qs * SB, (qs + 1) * SB)
        nlb = 4 * qs + 4          # key blocks needed (block-causal)
        # PE filler work popped between attention groups: the previous
        # superblock's (deferred) normalization + output projection, and the
        # (qs+2) projection chunk.
        # flush any overdue fillers (projections for THIS superblock)
        while fillers and fillers[0][0] <= qs:
            _, fn, args = fillers.pop(0)
            fn(*args)
        if prev_norm is not None:
            pdb, pdbb, paus = prev_norm
            for h in range(2):
                fillers.append((NQS, norm_mi, (qs - 1, h, pdb, pdbb, paus)))
            for sj in range(4):
                for dh in range(2):
                    fillers.append((NQS, wo_half, ((qs - 1) * 4 + sj, dh)))
        if qs == 0:
            fillers.extend((1, fn, a) for fn, a in proj_pieces(0)[7:])
            fillers.extend((1, fn, a) for fn, a in proj_pieces(1))
            fillers.append((1, lambda: nc.gpsimd.dma_start(wot_sb[:], wot_d),
                            ()))
        if qs + 2 < NQS:
            fillers.extend((qs + 2, fn, a) for fn, a in proj_pieces(qs + 2))
        # unnormalized attention rows + denominators for this superblock are
        # staged to SBUF immediately so the PV psum slots recycle fast and
        # normalization runs off the critical path (deferred into qs+1).
        db = ptmp.tile([32, 2, 2, SB], F32, tag="denom", name="db")
        dbb = ptmp.tile([32, 2, 2, SB], BF16, tag="denomb", name="dbb")
        aus = []

        for mi in range(2):       # head pair (2mi, 2mi+1) at partitions 0/64
            po = pp_pv.tile([HD + 1, 2, SB], F32, tag="pv", name="po")
            pes = {}

            def emit_pv(lb):
                j = lb - 4 * qs
                vcols = slice(max(j, 0) * 128, SB)
                for i in range(2):
                    nc.tensor.matmul(po[:, i, vcols], v_sb[:, lb, :],
                                     pes[lb][:, i, vcols],
                                     start=(lb == 0), stop=(lb == nlb - 1))
                del pes[lb]

            # S/exp run two groups ahead of PV, so each PV's exp result is
            # already in SBUF when the PE reaches it — the PE only ever
            # waits at sp-slot allocation (the intended ACT pacing point).
            for lb in range(nlb):
                j = lb - 4 * qs   # >=0 on the diagonal superblock
                kcols = slice(lb * 128, (lb + 1) * 128)
                sp = pp_sb.tile([128, 2, SB], F32, tag="sbig", name="sp")
                pe = pexp.tile([128, 2, SB], BF16, tag="pexp", name="pe")
                pes[lb] = pe
                if j < 0:
                    # the two matmuls run concurrently (row groups 0-1 / 2-3)
                    nc.tensor.matmul(sp[:, 0, :], kT_sb[0:64, kcols],
                                     qT_sb[0:64, mi, qcols],
                                     start=True, stop=True)
                    nc.tensor.matmul(sp[:, 1, :], kT_sb[64:128, kcols],
                                     qT_sb[64:128, mi, qcols],
                                     start=True, stop=True)
                    nc.scalar.activation(pe[:], sp[:], EXP, scale=SCALE)
                else:
                    ecols = slice(qs * SB + j * 128, (qs + 1) * SB)
                    scols = slice(j * 128, SB)
                    nc.tensor.matmul(sp[:, 0, scols], kT_sb[0:64, kcols],
                                     qT_sb[0:64, mi, ecols],
                                     start=True, stop=True)
                    nc.tensor.matmul(sp[:, 1, scols], kT_sb[64:128, kcols],
                                     qT_sb[64:128, mi, ecols],
                                     start=True, stop=True)
                    nc.scalar.activation(pe[:, :, scols], sp[:, :, scols],
                                         EXP, scale=SCALE)
                    dcols = slice(j * 128, (j + 1) * 128)
                    meng = nc.vector if j % 2 else nc.gpsimd
                    meng.tensor_mul(pe[:, :, dcols], pe[:, :, dcols],
                                    tri2_sb[:])
                if lb >= 2:
                    emit_pv(lb - 2)
                for _ in range(2 if qs < 2 else 1):
                    if fillers:
                        _, fn, args = fillers.pop(0)
                        fn(*args)
            emit_pv(nlb - 2)
            emit_pv(nlb - 1)
            au = ptmp.tile([128, SB], BF16, tag="aun", name="au")
            aus.append(au)
            # denominator staging runs on ScalarE (off the DVE critical
            # path); the custom-DVE reciprocal runs at base partition 0
            # (it writes garbage at other bases).  For the last (qs, mi)
            # there is no next PV waiting on the po slot, so the reciprocal
            # chain runs all-Vector and ahead of the au copies to shorten
            # the serial epilogue.
            last = qs == NQS - 1 and mi == 1
            if qs == NQS - 1:
                nc.vector.tensor_copy(db[0:1, mi, :, :], po[HD:HD + 1, :, :])
            else:
                nc.scalar.copy(db[0:1, mi, :, :], po[HD:HD + 1, :, :])
            nc.vector.reciprocal_approx_fast(db[0:1, mi, :, :],
                                             db[0:1, mi, :, :])
            nc.vector.tensor_copy(dbb[0:1, mi, :, :], db[0:1, mi, :, :])
            for i in range(2):
                b0 = i * 64
                aeng = nc.scalar if last else nc.vector
                aeng_copy = (nc.scalar.copy if last
                             else nc.vector.tensor_copy)
                aeng_copy(au[b0:b0 + 64, :], po[0:HD, i, :])
            if qs == NQS - 1 and mi == 0:
                # eager tail: normalize head-pair 0 while pair 1 computes
                fillers.append((NQS, norm_mi, (qs, 0, db, dbb, aus)))
        prev_norm = (db, dbb, aus)
    # epilogue: leftovers + last head-pair normalization + output proj
    for _, fn, args in fillers:
        fn(*args)
    pdb, pdbb, paus = prev_norm
    norm_mi(NQS - 1, 1, pdb, pdbb, paus)
    for sj in range(4):
        wo_half((NQS - 1) * 4 + sj, 0)
        wo_half((NQS - 1) * 4 + sj, 1)
    if dbg:
        nc.sync.dma_start(dbg["qT"], qT_sb[:])
        nc.sync.dma_start(dbg["kT"], kT_sb[:])
        nc.sync.dma_start(dbg["v"], v_sb[:])
        nc.sync.dma_start(dbg["att"], att_sb[:])
        nc.sync.dma_start(dbg["db"], pdb[:])


# ---------------------------------------------------------------- host side
def _prep_inputs(x, wq, wk, wv, wo, freqs_cos, freqs_sin):
    """Shard + lay out host-side. Returns list of 8 in_maps.

    Everything is packed per-SBUF-partition-contiguous so the input DMAs
    run as [128 x big-contiguous-row] transfers at full HBM bandwidth.
    """
    bf = ml_dtypes.bfloat16
    # even/odd pair permutation within each head's 64 rows
    perm = np.concatenate([np.arange(0, HD, 2), np.arange(1, HD, 2)])
    cos4 = np.tile(np.ascontiguousarray(freqs_cos.T), (4, 1))  # (128, S)
    sin4 = np.tile(np.ascontiguousarray(freqs_sin.T), (4, 1))
    cs = np.ascontiguousarray(np.stack([cos4, sin4], axis=1)).astype(bf)
    tri = np.triu(np.ones((128, 128), np.float32))  # [l, q]: l <= q
    tri2 = np.ascontiguousarray(np.stack([tri, tri], axis=1)).astype(bf)
    # rope pair-swap-with-sign permutation: (P2 @ m1)[r-rows] = -m1[i-rows],
    # (P2 @ m1)[i-rows] = +m1[r-rows]; stationary operand is P2^T
    p2 = np.zeros((128, 128), np.float32)
    for blk in range(2):
        b0 = blk * 64
        for j in range(32):
            p2[b0 + j, b0 + 32 + j] = -1.0
            p2[b0 + 32 + j, b0 + j] = 1.0
    p2t = np.ascontiguousarray(p2.T).astype(bf)

    in_maps = []
    for c in range(N_CORES):
        b, g = divmod(c, 4)
        xt = np.ascontiguousarray(
            x[b].T.reshape(KCH, 128, NQS, SB).transpose(1, 2, 0, 3)
        ).astype(bf)
        wq_g = wq[g * GO:(g + 1) * GO].reshape(NH, HD, D)[:, perm, :].reshape(GO, D)
        wqt = np.ascontiguousarray(
            wq_g.T.reshape(KCH, 128, GO).transpose(1, 0, 2)).astype(bf)
        wkt = wk[g * HD:(g + 1) * HD][perm].T
        wvt = wv[g * HD:(g + 1) * HD].T
        wkvt = np.ascontiguousarray(
            np.concatenate([wkt, wvt], 1).reshape(KCH, 128, 2 * HD)
            .transpose(1, 0, 2)).astype(bf)
        wot = np.ascontiguousarray(
            wo[:, g * GO:(g + 1) * GO].T.reshape(2, 128, D)
            .transpose(1, 0, 2)).astype(bf)
        in_maps.append({
            "xt": xt, "wqt": wqt, "wkvt": wkvt, "wot": wot,
            "cs": cs, "tri2": tri2, "p2t": p2t,
        })
    return in_maps


def get_nc():
    if "nc" not in _CACHE:
        _CACHE["nc"] = _build()
    return _CACHE["nc"]


def _ensure_ntff_hook():
    """The image's antenv lacks axon_hooks; inject an equivalent module so
    run_bass_kernel_spmd(trace=True) can capture NTFF profiles via the
    libaxon_pjrt.so C ABI (same shim trn_boot would register)."""
    import sys as _sys
    import types
    if "antenv.axon_hooks" in _sys.modules:
        return
    import contextlib
    import ctypes

    def _make_hook(so_path="/opt/axon/libaxon_pjrt.so"):
        try:
            lib = ctypes.CDLL(so_path)
        except OSError:
            return None
        if not hasattr(lib, "axon_start_nrt_profile"):
            return None
        lib.axon_start_nrt_profile.argtypes = [ctypes.POINTER(ctypes.c_int64),
                                               ctypes.c_size_t]
        lib.axon_start_nrt_profile.restype = ctypes.c_int64
        lib.axon_stop_nrt_profile.argtypes = [ctypes.c_char_p]
        lib.axon_stop_nrt_profile.restype = ctypes.c_int64

        @contextlib.contextmanager
        def _hook(output_dir, device_ids):
            import jax
            jax.devices()
            if device_ids:
                ids = (ctypes.c_int64 * len(device_ids))(*device_ids)
                rc = lib.axon_start_nrt_profile(ids, len(device_ids))
            else:
                rc = lib.axon_start_nrt_profile(None, 0)
            if rc != 0:
                raise RuntimeError(f"axon_start_nrt_profile rc={rc}")
            try:
                yield
            finally:
                n = lib.axon_stop_nrt_profile(str(output_dir).encode())
                print(f"profile: {n} file(s) -> {output_dir}", file=sys.stderr)

        return _hook

    hook = _make_hook()
    mod = types.ModuleType("antenv.axon_hooks")
    mod.get_axon_ntff_profile_hook = lambda: hook
    mod.set_axon_ntff_profile_hook = lambda h: None
    _sys.modules["antenv.axon_hooks"] = mod


def run(inputs, trace=False):
    from concourse.bass_utils import run_bass_kernel_spmd
    if trace:
        _ensure_ntff_hook()
    nc = get_nc()
    in_maps = _prep_inputs(**inputs)
    res = run_bass_kernel_spmd(nc, in_maps, core_ids=list(range(N_CORES)),
                               trace=trace)
    return res


def kernel(**inputs) -> np.ndarray:
    res = run(inputs)
    outs = [np.asarray(r["out"], np.float32) for r in res.results]
    y = np.stack([outs[4 * b] + outs[4 * b + 1] + outs[4 * b + 2] + outs[4 * b + 3]
                  for b in range(B)])
    return y.astype(np.float32)

